# revision 17
# baseline (speedup 1.0000x reference)
"""AttnLSTM Trainium2 kernel: enc -> LSTM(T=512) -> attention pool -> decode.

Sharding: data-parallel over batch B=1024 across 8 cores (128 batch/core).
Compute layout: H on partitions 0:32, batch on free dim; gates along free
(z = [32, 4gates, 128batch]) so every elementwise op is partition-base-0.
Gate order permuted to [i, f, o, g].
"""

import os
import sys

for p in ("/opt/trn_rl_repo", os.path.expanduser("~/.axon_site/_ro/trn_rl_repo")):
    if os.path.isdir(p) and p not in sys.path:
        sys.path.insert(0, p)

import numpy as np

import concourse.bass as bass
import concourse.mybir as mybir
import concourse.tile as tile
from concourse import bacc, bass_utils

B, T, F, H = 1024, 512, 64, 32
NCORES = 8
BS = B // NCORES  # 128
AF = mybir.ActivationFunctionType
DT = mybir.dt.float32
BF = mybir.dt.bfloat16

# gate permutation: keras [i f g o] -> ours [i f o g]
_PERM = np.concatenate([np.arange(0, 64), np.arange(96, 128), np.arange(64, 96)])

_CACHE = {}


def _build(Tn, abl=()):
    nc = bacc.Bacc("TRN2", debug=False, num_devices=NCORES)
    x_d = nc.dram_tensor("x", [BS, Tn, F], BF, kind="ExternalInput")
    wenc2_d = nc.dram_tensor("wenc2", [128, 128], BF, kind="ExternalInput")
    benc2_d = nc.dram_tensor("benc2", [128, 1], DT, kind="ExternalInput")
    kern2_d = nc.dram_tensor("kern2", [128, 4 * H], BF, kind="ExternalInput")
    rec_d = nc.dram_tensor("rec", [H, 4 * H], BF, kind="ExternalInput")
    b4t_d = nc.dram_tensor("b4t", [4, H], BF, kind="ExternalInput")
    bind4_d = nc.dram_tensor("bind4", [4, 4 * BS], BF, kind="ExternalInput")
    sel4_d = nc.dram_tensor("sel4", [H, 4, 128], BF, kind="ExternalInput")
    attnw4_d = nc.dram_tensor("attnw4", [128, 128], BF, kind="ExternalInput")
    attnb4_d = nc.dram_tensor("attnb4", [128, 1], DT, kind="ExternalInput")
    attnu4_d = nc.dram_tensor("attnu4", [128, 4], DT, kind="ExternalInput")
    grp_d = nc.dram_tensor("grp", [4, 128], BF, kind="ExternalInput")
    ones4_d = nc.dram_tensor("ones4", [4, 1], BF, kind="ExternalInput")
    ones32_d = nc.dram_tensor("ones32", [1, H], DT, kind="ExternalInput")
    grpsel_d = nc.dram_tensor("grpsel", [128, H], DT, kind="ExternalInput")
    wdec0_d = nc.dram_tensor("wdec0", [H, 1], BF, kind="ExternalInput")
    wdec1_d = nc.dram_tensor("wdec1", [H, 1], DT, kind="ExternalInput")
    bdec_d = nc.dram_tensor("bdec", [BS, 1], DT, kind="ExternalInput")
    ident_d = nc.dram_tensor("ident", [128, 128], DT, kind="ExternalInput")
    out_d = nc.dram_tensor("out", [BS, 1], DT, kind="ExternalOutput")

    TC = 16  # timesteps per chunk
    nchunks = Tn // TC
    NQ = Tn // 4  # quads

    with tile.TileContext(nc) as tc:
        from contextlib import ExitStack

        ctx = ExitStack()
        with ctx:
            constp = ctx.enter_context(tc.tile_pool(name="const", bufs=1))
            xinp = ctx.enter_context(tc.tile_pool(name="xin", bufs=3))
            xtp = ctx.enter_context(tc.tile_pool(name="xt", bufs=3))
            encp = ctx.enter_context(tc.tile_pool(name="enc", bufs=3))
            sgp = ctx.enter_context(tc.tile_pool(name="sg", bufs=3))
            smp = ctx.enter_context(tc.tile_pool(name="sm", bufs=1))
            trps = ctx.enter_context(tc.tile_pool(name="trps", bufs=1, space="PSUM"))
            encps = ctx.enter_context(tc.tile_pool(name="encps", bufs=2, space="PSUM"))
            zps = ctx.enter_context(tc.tile_pool(name="zps", bufs=3, space="PSUM"))
            hqps = ctx.enter_context(tc.tile_pool(name="hqps", bufs=1, space="PSUM"))
            attps = ctx.enter_context(tc.tile_pool(name="attps", bufs=1, space="PSUM"))

            # ---- constants ----
            def cload(name, shape, dtype, src):
                t = constp.tile(shape, dtype, name=name)
                nc.sync.dma_start(t[:], src[tuple(slice(None) for _ in shape)])
                return t

            ident = cload("ident", [128, 128], DT, ident_d)
            wenc2 = cload("wenc2", [128, 128], BF, wenc2_d)
            benc2 = cload("benc2", [128, 1], DT, benc2_d)
            kern2 = cload("kern2", [128, 4 * H], BF, kern2_d)
            rec = cload("rec", [H, 4 * H], BF, rec_d)
            b4t = cload("b4t", [4, H], BF, b4t_d)
            bind4 = cload("bind4", [4, 4 * BS], BF, bind4_d)
            sel4 = cload("sel4", [H, 4, 128], BF, sel4_d)
            attnw4 = cload("attnw4", [128, 128], BF, attnw4_d)
            attnb4 = cload("attnb4", [128, 1], DT, attnb4_d)
            attnu4 = cload("attnu4", [128, 4], DT, attnu4_d)
            grp = cload("grp", [4, 128], BF, grp_d)
            ones4 = cload("ones4", [4, 1], BF, ones4_d)
            ones32 = cload("ones32", [1, H], DT, ones32_d)
            grpsel = cload("grpsel", [128, H], DT, grpsel_d)
            wdec0 = cload("wdec0", [H, 1], BF, wdec0_d)
            wdec1 = cload("wdec1", [H, 1], DT, wdec1_d)
            bdec = cload("bdec", [BS, 1], DT, bdec_d)

            # ---- persistent state ----
            h_cur = constp.tile([H, BS], BF)
            c_st = constp.tile([H, BS], DT)
            m_t = constp.tile([H, BS], BF)
            v_t = constp.tile([H, BS], DT)
            th_t = constp.tile([H, BS], BF)
            hstore = constp.tile([128, NQ * 128], BF)  # quad-packed h history
            lstore = constp.tile([4, NQ * 128], BF)  # attention logits
            acc = constp.tile([128, BS], DT)  # pooled accum (4-way split)
            nc.any.memset(h_cur[:], 0.0)
            nc.any.memset(c_st[:], 0.0)
            nc.any.memset(acc[:], 0.0)

            ztiles = {}
            hq_ps = {}
            encsb = {}

            def emit_dma(ck):
                xin = xinp.tile([128, TC, F], BF, tag="xin", name="xin")
                nc.sync.dma_start(xin[:], x_d[:, ck * TC : (ck + 1) * TC, :])
                return xin

            def emit_pre(xin, t):
                # pairs: DMA-transpose x[t,t+1] -> [128(2t,f), 128b]; blockdiag enc
                if t % 2 == 1:
                    return
                j = t % TC
                xts = xtp.tile([128, 128], BF, tag="xt", name="xts")
                nc.sync.dma_start_transpose(xts[:], xin[:, j : j + 2, :])
                eps = encps.tile([128, BS], DT, tag="encps", name="encps")
                nc.tensor.matmul(eps[:], wenc2[:], xts[:], start=True, stop=True,
                                 skip_group_check=True)
                eb = encp.tile([128, BS], BF, tag="enc", name="encsb")
                nc.scalar.activation(eb[:], eps[:], AF.Tanh, bias=benc2[:])
                encsb[t] = eb[0:64, :]
                encsb[t + 1] = eb[64:128, :]

            def emit_xg(t):
                # z psum tile [32, 4, 128]; bias + input-gate contributions
                zt = zps.tile([H, 4, BS], DT, tag="z", name="zt")
                ztiles[t] = zt
                nc.tensor.matmul(zt[:], b4t[:], bind4[:], start=True, stop=False,
                                 skip_group_check=True)
                eb = encsb.pop(t)
                kb = 64 * (t % 2)
                for g in range(4):
                    nc.tensor.matmul(
                        zt[:, g, :],
                        kern2[kb : kb + 64, 32 * g : 32 * (g + 1)],
                        eb,
                        start=False,
                        stop=False,
                        skip_group_check=True,
                    )

            def step(t):
                q, j = divmod(t, 4)
                zt = ztiles.pop(t)
                # recurrence: g gate first so tanh can start early
                for g in (() if "norec" in abl else (0, 1, 2, 3)):
                    nc.tensor.matmul(
                        zt[:, g, :],
                        rec[:, 32 * g : 32 * (g + 1)],
                        h_cur[:],
                        start=False,
                        stop=(g == 3),
                        skip_group_check=True,
                    )
                s = sgp.tile([H, 4, BS], BF, tag="s", name="sgate")
                nc.scalar.activation(s[:], zt[:], AF.Sigmoid)
                # tanh(g) = 2*sigmoid(2g) - 1; g-gate weights pre-doubled.
                # m = i*tanh(g) = (2*s_g)*s_i - s_i
                nc.vector.scalar_tensor_tensor(
                    m_t[:], s[:, 3, :], 2.0, s[:, 0, :],
                    mybir.AluOpType.mult, mybir.AluOpType.mult,
                )
                nc.vector.tensor_sub(m_t[:], m_t[:], s[:, 0, :])
                nc.vector.tensor_mul(v_t[:], s[:, 1, :], c_st[:])
                nc.vector.tensor_add(c_st[:], m_t[:], v_t[:])
                if "noth" not in abl:
                    nc.scalar.activation(th_t[:], c_st[:], AF.Tanh)
                    nc.vector.tensor_mul(h_cur[:], s[:, 2, :], th_t[:])
                # pack h into quad psum via selector matmul
                if "nohq" in abl:
                    return
                if j == 0:
                    hq_ps[q] = hqps.tile([128, 128], DT, tag="hq", name="hq")
                nc.tensor.matmul(
                    hq_ps[q][:],
                    sel4[:, j, :],
                    h_cur[:],
                    start=(j == 0),
                    stop=(j == 3),
                    skip_group_check=True,
                )

            def pool_quad(q):
                hq = hq_ps.pop(q)
                hsl = hstore[:, 128 * q : 128 * (q + 1)]
                nc.vector.tensor_copy(hsl, hq[:])
                lps = attps.tile([128, 128], DT, tag="att", name="latps")
                nc.tensor.matmul(lps[:], attnw4[:], hsl, start=True, stop=True,
                                 skip_group_check=True)
                lat = sgp.tile([128, BS], DT, tag="lat", name="lat")
                nc.scalar.activation(lat[:], lps[:], AF.Tanh, bias=attnb4[:])
                gps = attps.tile([4, BS], DT, tag="att", name="gps")
                nc.tensor.matmul(gps[:], attnu4[:], lat[:], start=True, stop=True,
                                 skip_group_check=True)
                nc.vector.tensor_copy(lstore[:, 128 * q : 128 * (q + 1)], gps[:])

            # ---- main pipeline ----
            xin_cur = emit_dma(0)
            for j in range(TC):
                emit_pre(xin_cur, j)
            for t in range(4):
                emit_xg(t)
            xin_nxt = emit_dma(1) if nchunks > 1 else None

            for ck in range(nchunks):
                for j in range(TC):
                    t = ck * TC + j
                    step(t)
                    if t + 4 < Tn:
                        emit_xg(t + 4)
                    if t % 4 == 3 and "nopool" not in abl:
                        pool_quad(t // 4)
                    # phase A of chunk ck+1, spread across this chunk's steps
                    if ck + 1 < nchunks:
                        emit_pre(xin_nxt, (ck + 1) * TC + j)
                        if j == TC - 1:
                            xin_cur = xin_nxt
                            xin_nxt = emit_dma(ck + 2) if ck + 2 < nchunks else None

            # ---- post-pass: softmax pooling + decode ----
            estore = constp.tile([4, NQ * 128], BF)
            nc.scalar.activation(estore[:], lstore[:], AF.Exp)
            seps = encps.tile([1, BS], DT, tag="encps", name="seps")
            for q in range(NQ):
                esl = estore[:, 128 * q : 128 * (q + 1)]
                nc.tensor.matmul(
                    seps[:], ones4[:], esl, start=(q == 0), stop=(q == NQ - 1),
                    skip_group_check=True,
                )
                ebc = trps.tile([128, BS], DT, tag="tr", name="ebc")
                nc.tensor.matmul(ebc[:], grp[:], esl, start=True, stop=True,
                                 skip_group_check=True)
                wx = sgp.tile([128, BS], DT, tag="wx", name="wx")
                nc.vector.tensor_mul(wx[:], ebc[:], hstore[:, 128 * q : 128 * (q + 1)])
                nc.vector.tensor_add(acc[:], acc[:], wx[:])

            # pooled = (sum_g acc) / se ; out = sigmoid(h.Wd0 + pooled.Wd1 + b)
            pfold = attps.tile([H, BS], DT, tag="att", name="pfold")
            nc.tensor.matmul(pfold[:], grpsel[:], acc[:], start=True, stop=True,
                             skip_group_check=True)
            rse = smp.tile([1, BS], DT, name="rse")
            nc.vector.reciprocal(rse[:], seps[:])
            rbc = trps.tile([H, BS], DT, tag="tr", name="rbc")
            nc.tensor.matmul(rbc[:], ones32[:], rse[:], start=True, stop=True,
                             skip_group_check=True)
            rbcs = smp.tile([H, BS], DT, name="rbcs")
            nc.vector.tensor_copy(rbcs[:], rbc[:])
            pooled = smp.tile([H, BS], DT, name="pooled")
            nc.vector.tensor_mul(pooled[:], pfold[:], rbcs[:])
            po = encps.tile([BS, 1], DT, tag="encps", name="po")
            nc.tensor.matmul(po[:], h_cur[:], wdec0[:], start=True, stop=False,
                             skip_group_check=True)
            nc.tensor.matmul(po[:], pooled[:], wdec1[:], start=False,
                             stop=True, skip_group_check=True)
            osb = smp.tile([BS, 1], DT, name="osb")
            nc.scalar.activation(osb[:], po[:], AF.Sigmoid, bias=bdec[:])
            nc.sync.dma_start(out_d[:, :], osb[:])

    nc.compile()
    return nc


def _prep_shared(W_enc, b_enc, kernel, recurrent, bias, attn_W, attn_b, attn_u,
                 W_dec, b_dec):
    import ml_dtypes
    f32 = np.float32
    bf16 = ml_dtypes.bfloat16

    wenc2 = np.zeros((128, 128), np.float32)
    wenc2[0:64, 0:64] = W_enc
    wenc2[64:128, 64:128] = W_enc
    gscale = np.ones(128, np.float32)
    gscale[96:128] = 2.0  # tanh(g) = 2*sigmoid(2g)-1
    kern = np.ascontiguousarray(kernel[:, _PERM] * gscale).astype(f32)
    recp = np.ascontiguousarray(recurrent[:, _PERM] * gscale).astype(f32)
    biasp = (bias[_PERM] * gscale).astype(f32)  # [128] in [i f o g] order
    b4t = np.ascontiguousarray(biasp.reshape(4, H))  # [gate, h]
    bind4 = np.zeros((4, 4 * BS), f32)
    for g in range(4):
        bind4[g, g * BS : (g + 1) * BS] = 1.0
    sel4 = np.zeros((H, 4, 128), f32)
    for j in range(4):
        sel4[:, j, 32 * j : 32 * (j + 1)] = np.eye(H)
    attnw4 = np.zeros((128, 128), f32)
    attnb4 = np.zeros((128, 1), f32)
    attnu4 = np.zeros((128, 4), f32)
    for g in range(4):
        attnw4[32 * g : 32 * (g + 1), 32 * g : 32 * (g + 1)] = attn_W
        attnb4[32 * g : 32 * (g + 1), 0] = attn_b
        attnu4[32 * g : 32 * (g + 1), g] = attn_u
    grp = np.zeros((4, 128), f32)
    for g in range(4):
        grp[g, 32 * g : 32 * (g + 1)] = 1.0
    grpsel = np.zeros((128, H), f32)
    for g in range(4):
        grpsel[32 * g : 32 * (g + 1), :] = np.eye(H)
    return {
        "wenc2": wenc2.astype(bf16),
        "benc2": np.concatenate([b_enc, b_enc]).reshape(128, 1).astype(f32),
        "kern2": np.vstack([kern, kern]).astype(bf16),
        "rec": recp.astype(bf16),
        "b4t": b4t.astype(bf16),
        "bind4": bind4.astype(bf16),
        "sel4": sel4.astype(bf16),
        "attnw4": attnw4.astype(bf16),
        "attnb4": attnb4,
        "attnu4": attnu4,
        "grp": grp.astype(bf16),
        "ones4": np.ones((4, 1), f32).astype(bf16),
        "ones32": np.ones((1, H), f32),
        "grpsel": grpsel,
        "wdec0": np.ascontiguousarray(W_dec.astype(f32).reshape(2 * H, 1)[0:H]).astype(bf16),
        "wdec1": np.ascontiguousarray(W_dec.astype(f32).reshape(2 * H, 1)[H:]),
        "bdec": np.full((BS, 1), float(np.asarray(b_dec).reshape(-1)[0]), f32),
        "ident": np.eye(128, dtype=f32),
    }


def kernel(x, W_enc, b_enc, kernel, recurrent, bias, attn_W, attn_b, attn_u,
           W_dec, b_dec, _trace=False):
    import ml_dtypes
    x = np.asarray(x, np.float32).astype(ml_dtypes.bfloat16)
    Tn = x.shape[1]
    shared = _prep_shared(
        np.asarray(W_enc), np.asarray(b_enc), np.asarray(kernel),
        np.asarray(recurrent), np.asarray(bias), np.asarray(attn_W),
        np.asarray(attn_b), np.asarray(attn_u), np.asarray(W_dec),
        np.asarray(b_dec),
    )
    if Tn not in _CACHE:
        _CACHE[Tn] = _build(Tn)
    nc = _CACHE[Tn]
    in_maps = []
    for c in range(NCORES):
        m = dict(shared)
        m["x"] = np.ascontiguousarray(x[c * BS : (c + 1) * BS])
        in_maps.append(m)
    res = bass_utils.run_bass_kernel_spmd(
        nc, in_maps, core_ids=list(range(NCORES)), trace=_trace
    )
    out = np.concatenate([res.results[c]["out"] for c in range(NCORES)], axis=0)
    globals()["_LAST_EXEC_NS"] = getattr(res, "exec_time_ns", None)
    return out


# revision 20
# speedup vs baseline: 1.0513x; 1.0513x over previous
"""AttnLSTM Trainium2 kernel: enc -> LSTM(T=512) -> attention pool -> decode.

Sharding: data-parallel over batch B=1024 across 8 cores (128 batch/core).
Compute layout: H on partitions 0:32, batch on free dim; gates along free
(z = [32, 4gates, 128batch]) so every elementwise op is partition-base-0.
Gate order permuted to [i, f, o, g].
"""

import os
import sys

for p in ("/opt/trn_rl_repo", os.path.expanduser("~/.axon_site/_ro/trn_rl_repo")):
    if os.path.isdir(p) and p not in sys.path:
        sys.path.insert(0, p)

import numpy as np

import concourse.bass as bass
import concourse.mybir as mybir
import concourse.tile as tile
from concourse import bacc, bass_utils

B, T, F, H = 1024, 512, 64, 32
NCORES = 8
BS = B // NCORES  # 128
AF = mybir.ActivationFunctionType
DT = mybir.dt.float32
BF = mybir.dt.bfloat16

# gate permutation: keras [i f g o] -> ours [i f o g]
_PERM = np.concatenate([np.arange(0, 64), np.arange(96, 128), np.arange(64, 96)])

_CACHE = {}


def _build(Tn, abl=()):
    nc = bacc.Bacc("TRN2", debug=False, num_devices=NCORES)
    x_d = nc.dram_tensor("x", [BS, Tn, F], BF, kind="ExternalInput")
    wenc2_d = nc.dram_tensor("wenc2", [128, 128], BF, kind="ExternalInput")
    benc2_d = nc.dram_tensor("benc2", [128, 1], DT, kind="ExternalInput")
    kern2_d = nc.dram_tensor("kern2", [128, 4 * H], BF, kind="ExternalInput")
    rec_d = nc.dram_tensor("rec", [H, 4 * H], BF, kind="ExternalInput")
    b4t_d = nc.dram_tensor("b4t", [4, H], BF, kind="ExternalInput")
    bind4_d = nc.dram_tensor("bind4", [4, 4 * BS], BF, kind="ExternalInput")
    sel4_d = nc.dram_tensor("sel4", [H, 4, 128], BF, kind="ExternalInput")
    attnw4_d = nc.dram_tensor("attnw4", [128, 128], BF, kind="ExternalInput")
    attnb4_d = nc.dram_tensor("attnb4", [128, 1], DT, kind="ExternalInput")
    attnu4_d = nc.dram_tensor("attnu4", [128, 4], DT, kind="ExternalInput")
    grp_d = nc.dram_tensor("grp", [4, 128], BF, kind="ExternalInput")
    ones4_d = nc.dram_tensor("ones4", [4, 1], BF, kind="ExternalInput")
    ones32_d = nc.dram_tensor("ones32", [1, H], DT, kind="ExternalInput")
    grpsel_d = nc.dram_tensor("grpsel", [128, H], DT, kind="ExternalInput")
    wdec0_d = nc.dram_tensor("wdec0", [H, 1], BF, kind="ExternalInput")
    wdec1_d = nc.dram_tensor("wdec1", [H, 1], DT, kind="ExternalInput")
    bdec_d = nc.dram_tensor("bdec", [BS, 1], DT, kind="ExternalInput")
    ident_d = nc.dram_tensor("ident", [128, 128], DT, kind="ExternalInput")
    out_d = nc.dram_tensor("out", [BS, 1], DT, kind="ExternalOutput")

    TC = 16  # timesteps per chunk
    nchunks = Tn // TC
    NQ = Tn // 4  # quads

    with tile.TileContext(nc) as tc:
        from contextlib import ExitStack

        ctx = ExitStack()
        with ctx:
            constp = ctx.enter_context(tc.tile_pool(name="const", bufs=1))
            xinp = ctx.enter_context(tc.tile_pool(name="xin", bufs=3))
            xtp = ctx.enter_context(tc.tile_pool(name="xt", bufs=3))
            encp = ctx.enter_context(tc.tile_pool(name="enc", bufs=3))
            sgp = ctx.enter_context(tc.tile_pool(name="sg", bufs=3))
            smp = ctx.enter_context(tc.tile_pool(name="sm", bufs=1))
            trps = ctx.enter_context(tc.tile_pool(name="trps", bufs=1, space="PSUM"))
            encps = ctx.enter_context(tc.tile_pool(name="encps", bufs=2, space="PSUM"))
            zps = ctx.enter_context(tc.tile_pool(name="zps", bufs=3, space="PSUM"))
            hqps = ctx.enter_context(tc.tile_pool(name="hqps", bufs=1, space="PSUM"))
            attps = ctx.enter_context(tc.tile_pool(name="attps", bufs=1, space="PSUM"))

            # ---- constants ----
            def cload(name, shape, dtype, src):
                t = constp.tile(shape, dtype, name=name)
                nc.sync.dma_start(t[:], src[tuple(slice(None) for _ in shape)])
                return t

            ident = cload("ident", [128, 128], DT, ident_d)
            wenc2 = cload("wenc2", [128, 128], BF, wenc2_d)
            benc2 = cload("benc2", [128, 1], DT, benc2_d)
            kern2 = cload("kern2", [128, 4 * H], BF, kern2_d)
            rec = cload("rec", [H, 4 * H], BF, rec_d)
            b4t = cload("b4t", [4, H], BF, b4t_d)
            bind4 = cload("bind4", [4, 4 * BS], BF, bind4_d)
            sel4 = cload("sel4", [H, 4, 128], BF, sel4_d)
            attnw4 = cload("attnw4", [128, 128], BF, attnw4_d)
            attnb4 = cload("attnb4", [128, 1], DT, attnb4_d)
            attnu4 = cload("attnu4", [128, 4], DT, attnu4_d)
            grp = cload("grp", [4, 128], BF, grp_d)
            ones4 = cload("ones4", [4, 1], BF, ones4_d)
            ones32 = cload("ones32", [1, H], DT, ones32_d)
            grpsel = cload("grpsel", [128, H], DT, grpsel_d)
            wdec0 = cload("wdec0", [H, 1], BF, wdec0_d)
            wdec1 = cload("wdec1", [H, 1], DT, wdec1_d)
            bdec = cload("bdec", [BS, 1], DT, bdec_d)

            # ---- persistent state ----
            h_cur = constp.tile([H, BS], BF)
            c_st = constp.tile([H, BS], BF)
            m_t = constp.tile([H, BS], BF)
            v_t = constp.tile([H, BS], BF)
            th_t = constp.tile([H, BS], BF)
            hstore = constp.tile([128, NQ * 128], BF)  # quad-packed h history
            lstore = constp.tile([4, NQ * 128], BF)  # attention logits
            acc = constp.tile([128, BS], DT)  # pooled accum (4-way split)
            nc.any.memset(h_cur[:], 0.0)
            nc.any.memset(c_st[:], 0.0)
            nc.any.memset(acc[:], 0.0)

            ztiles = {}
            hq_ps = {}
            encsb = {}

            def emit_dma(ck):
                xin = xinp.tile([128, TC, F], BF, tag="xin", name="xin")
                nc.sync.dma_start(xin[:], x_d[:, ck * TC : (ck + 1) * TC, :])
                return xin

            def emit_pre(xin, t):
                # pairs: DMA-transpose x[t,t+1] -> [128(2t,f), 128b]; blockdiag enc
                if t % 2 == 1:
                    return
                j = t % TC
                xts = xtp.tile([128, 128], BF, tag="xt", name="xts")
                nc.sync.dma_start_transpose(xts[:], xin[:, j : j + 2, :])
                eps = encps.tile([128, BS], DT, tag="encps", name="encps")
                nc.tensor.matmul(eps[:], wenc2[:], xts[:], start=True, stop=True,
                                 skip_group_check=True)
                eb = encp.tile([128, BS], BF, tag="enc", name="encsb")
                nc.scalar.activation(eb[:], eps[:], AF.Tanh, bias=benc2[:])
                encsb[t] = eb[0:64, :]
                encsb[t + 1] = eb[64:128, :]

            def emit_xg(t):
                # z psum tile [32, 4, 128]; bias + input-gate contributions
                zt = zps.tile([H, 4, BS], DT, tag="z", name="zt")
                ztiles[t] = zt
                nc.tensor.matmul(zt[:], b4t[:], bind4[:], start=True, stop=False,
                                 skip_group_check=True)
                eb = encsb.pop(t)
                kb = 64 * (t % 2)
                for g in range(4):
                    nc.tensor.matmul(
                        zt[:, g, :],
                        kern2[kb : kb + 64, 32 * g : 32 * (g + 1)],
                        eb,
                        start=False,
                        stop=False,
                        skip_group_check=True,
                    )

            def step(t):
                q, j = divmod(t, 4)
                zt = ztiles.pop(t)
                # recurrence: g gate first so tanh can start early
                for g in (() if "norec" in abl else (0, 1, 2, 3)):
                    nc.tensor.matmul(
                        zt[:, g, :],
                        rec[:, 32 * g : 32 * (g + 1)],
                        h_cur[:],
                        start=False,
                        stop=(g == 3),
                        skip_group_check=True,
                    )
                s = sgp.tile([H, 4, BS], BF, tag="s", name="sgate")
                nc.scalar.activation(s[:], zt[:], AF.Sigmoid)
                # tanh(g) = 2*sigmoid(2g) - 1; g-gate weights pre-doubled.
                # m = i*tanh(g) = (2*s_g)*s_i - s_i
                tg = sgp.tile([H, BS], BF, tag="tg", name="tg")
                nc.vector.tensor_scalar(
                    tg[:], s[:, 3, :], 2.0, -1.0,
                    op0=mybir.AluOpType.mult, op1=mybir.AluOpType.add,
                )
                nc.vector.tensor_mul(m_t[:], tg[:], s[:, 0, :])
                nc.vector.tensor_mul(v_t[:], s[:, 1, :], c_st[:])
                nc.vector.tensor_add(c_st[:], m_t[:], v_t[:])
                if "noth" not in abl:
                    nc.scalar.activation(th_t[:], c_st[:], AF.Tanh)
                    nc.vector.tensor_mul(h_cur[:], s[:, 2, :], th_t[:])
                # pack h into quad psum via selector matmul
                if "nohq" in abl:
                    return
                if j == 0:
                    hq_ps[q] = hqps.tile([128, 128], DT, tag="hq", name="hq")
                nc.tensor.matmul(
                    hq_ps[q][:],
                    sel4[:, j, :],
                    h_cur[:],
                    start=(j == 0),
                    stop=(j == 3),
                    skip_group_check=True,
                )

            def pool_quad(q):
                hq = hq_ps.pop(q)
                hsl = hstore[:, 128 * q : 128 * (q + 1)]
                nc.vector.tensor_copy(hsl, hq[:])
                lps = attps.tile([128, 128], DT, tag="att", name="latps")
                nc.tensor.matmul(lps[:], attnw4[:], hsl, start=True, stop=True,
                                 skip_group_check=True)
                lat = sgp.tile([128, BS], DT, tag="lat", name="lat")
                nc.scalar.activation(lat[:], lps[:], AF.Tanh, bias=attnb4[:])
                gps = attps.tile([4, BS], DT, tag="att", name="gps")
                nc.tensor.matmul(gps[:], attnu4[:], lat[:], start=True, stop=True,
                                 skip_group_check=True)
                nc.vector.tensor_copy(lstore[:, 128 * q : 128 * (q + 1)], gps[:])

            # ---- main pipeline ----
            xin_cur = emit_dma(0)
            for j in range(TC):
                emit_pre(xin_cur, j)
            for t in range(4):
                emit_xg(t)
            xin_nxt = emit_dma(1) if nchunks > 1 else None

            for ck in range(nchunks):
                for j in range(TC):
                    t = ck * TC + j
                    step(t)
                    if t + 4 < Tn:
                        emit_xg(t + 4)
                    if t % 4 == 3 and "nopool" not in abl:
                        pool_quad(t // 4)
                    # phase A of chunk ck+1, spread across this chunk's steps
                    if ck + 1 < nchunks:
                        emit_pre(xin_nxt, (ck + 1) * TC + j)
                        if j == TC - 1:
                            xin_cur = xin_nxt
                            xin_nxt = emit_dma(ck + 2) if ck + 2 < nchunks else None

            # ---- post-pass: softmax pooling + decode ----
            estore = constp.tile([4, NQ * 128], BF)
            nc.scalar.activation(estore[:], lstore[:], AF.Exp)
            seps = encps.tile([1, BS], DT, tag="encps", name="seps")
            for q in range(NQ):
                esl = estore[:, 128 * q : 128 * (q + 1)]
                nc.tensor.matmul(
                    seps[:], ones4[:], esl, start=(q == 0), stop=(q == NQ - 1),
                    skip_group_check=True,
                )
                ebc = trps.tile([128, BS], DT, tag="tr", name="ebc")
                nc.tensor.matmul(ebc[:], grp[:], esl, start=True, stop=True,
                                 skip_group_check=True)
                wx = sgp.tile([128, BS], DT, tag="wx", name="wx")
                nc.vector.tensor_mul(wx[:], ebc[:], hstore[:, 128 * q : 128 * (q + 1)])
                nc.vector.tensor_add(acc[:], acc[:], wx[:])

            # pooled = (sum_g acc) / se ; out = sigmoid(h.Wd0 + pooled.Wd1 + b)
            pfold = attps.tile([H, BS], DT, tag="att", name="pfold")
            nc.tensor.matmul(pfold[:], grpsel[:], acc[:], start=True, stop=True,
                             skip_group_check=True)
            rse = smp.tile([1, BS], DT, name="rse")
            nc.vector.reciprocal(rse[:], seps[:])
            rbc = trps.tile([H, BS], DT, tag="tr", name="rbc")
            nc.tensor.matmul(rbc[:], ones32[:], rse[:], start=True, stop=True,
                             skip_group_check=True)
            rbcs = smp.tile([H, BS], DT, name="rbcs")
            nc.vector.tensor_copy(rbcs[:], rbc[:])
            pooled = smp.tile([H, BS], DT, name="pooled")
            nc.vector.tensor_mul(pooled[:], pfold[:], rbcs[:])
            po = encps.tile([BS, 1], DT, tag="encps", name="po")
            nc.tensor.matmul(po[:], h_cur[:], wdec0[:], start=True, stop=False,
                             skip_group_check=True)
            nc.tensor.matmul(po[:], pooled[:], wdec1[:], start=False,
                             stop=True, skip_group_check=True)
            osb = smp.tile([BS, 1], DT, name="osb")
            nc.scalar.activation(osb[:], po[:], AF.Sigmoid, bias=bdec[:])
            nc.sync.dma_start(out_d[:, :], osb[:])

    nc.compile()
    return nc


def _prep_shared(W_enc, b_enc, kernel, recurrent, bias, attn_W, attn_b, attn_u,
                 W_dec, b_dec):
    import ml_dtypes
    f32 = np.float32
    bf16 = ml_dtypes.bfloat16

    wenc2 = np.zeros((128, 128), np.float32)
    wenc2[0:64, 0:64] = W_enc
    wenc2[64:128, 64:128] = W_enc
    gscale = np.ones(128, np.float32)
    gscale[96:128] = 2.0  # tanh(g) = 2*sigmoid(2g)-1
    kern = np.ascontiguousarray(kernel[:, _PERM] * gscale).astype(f32)
    recp = np.ascontiguousarray(recurrent[:, _PERM] * gscale).astype(f32)
    biasp = (bias[_PERM] * gscale).astype(f32)  # [128] in [i f o g] order
    b4t = np.ascontiguousarray(biasp.reshape(4, H))  # [gate, h]
    bind4 = np.zeros((4, 4 * BS), f32)
    for g in range(4):
        bind4[g, g * BS : (g + 1) * BS] = 1.0
    sel4 = np.zeros((H, 4, 128), f32)
    for j in range(4):
        sel4[:, j, 32 * j : 32 * (j + 1)] = np.eye(H)
    attnw4 = np.zeros((128, 128), f32)
    attnb4 = np.zeros((128, 1), f32)
    attnu4 = np.zeros((128, 4), f32)
    for g in range(4):
        attnw4[32 * g : 32 * (g + 1), 32 * g : 32 * (g + 1)] = attn_W
        attnb4[32 * g : 32 * (g + 1), 0] = attn_b
        attnu4[32 * g : 32 * (g + 1), g] = attn_u
    grp = np.zeros((4, 128), f32)
    for g in range(4):
        grp[g, 32 * g : 32 * (g + 1)] = 1.0
    grpsel = np.zeros((128, H), f32)
    for g in range(4):
        grpsel[32 * g : 32 * (g + 1), :] = np.eye(H)
    return {
        "wenc2": wenc2.astype(bf16),
        "benc2": np.concatenate([b_enc, b_enc]).reshape(128, 1).astype(f32),
        "kern2": np.vstack([kern, kern]).astype(bf16),
        "rec": recp.astype(bf16),
        "b4t": b4t.astype(bf16),
        "bind4": bind4.astype(bf16),
        "sel4": sel4.astype(bf16),
        "attnw4": attnw4.astype(bf16),
        "attnb4": attnb4,
        "attnu4": attnu4,
        "grp": grp.astype(bf16),
        "ones4": np.ones((4, 1), f32).astype(bf16),
        "ones32": np.ones((1, H), f32),
        "grpsel": grpsel,
        "wdec0": np.ascontiguousarray(W_dec.astype(f32).reshape(2 * H, 1)[0:H]).astype(bf16),
        "wdec1": np.ascontiguousarray(W_dec.astype(f32).reshape(2 * H, 1)[H:]),
        "bdec": np.full((BS, 1), float(np.asarray(b_dec).reshape(-1)[0]), f32),
        "ident": np.eye(128, dtype=f32),
    }


def kernel(x, W_enc, b_enc, kernel, recurrent, bias, attn_W, attn_b, attn_u,
           W_dec, b_dec, _trace=False):
    import ml_dtypes
    x = np.asarray(x, np.float32).astype(ml_dtypes.bfloat16)
    Tn = x.shape[1]
    shared = _prep_shared(
        np.asarray(W_enc), np.asarray(b_enc), np.asarray(kernel),
        np.asarray(recurrent), np.asarray(bias), np.asarray(attn_W),
        np.asarray(attn_b), np.asarray(attn_u), np.asarray(W_dec),
        np.asarray(b_dec),
    )
    if Tn not in _CACHE:
        _CACHE[Tn] = _build(Tn)
    nc = _CACHE[Tn]
    in_maps = []
    for c in range(NCORES):
        m = dict(shared)
        m["x"] = np.ascontiguousarray(x[c * BS : (c + 1) * BS])
        in_maps.append(m)
    res = bass_utils.run_bass_kernel_spmd(
        nc, in_maps, core_ids=list(range(NCORES)), trace=_trace
    )
    out = np.concatenate([res.results[c]["out"] for c in range(NCORES)], axis=0)
    globals()["_LAST_EXEC_NS"] = getattr(res, "exec_time_ns", None)
    return out


# revision 24
# speedup vs baseline: 1.0791x; 1.0264x over previous
"""AttnLSTM Trainium2 kernel: enc -> LSTM(T=512) -> attention pool -> decode.

Sharding: data-parallel over batch B=1024 across 8 cores (128 batch/core).
Compute layout: H on partitions 0:32, batch on free dim; gates along free
(z = [32, 4gates, 128batch]) so every elementwise op is partition-base-0.
Gate order permuted to [i, f, o, g].
"""

import os
import sys

for p in ("/opt/trn_rl_repo", os.path.expanduser("~/.axon_site/_ro/trn_rl_repo")):
    if os.path.isdir(p) and p not in sys.path:
        sys.path.insert(0, p)

import numpy as np

import concourse.bass as bass
import concourse.mybir as mybir
import concourse.tile as tile
from concourse import bacc, bass_utils

B, T, F, H = 1024, 512, 64, 32
NCORES = 8
BS = B // NCORES  # 128
AF = mybir.ActivationFunctionType
DT = mybir.dt.float32
BF = mybir.dt.bfloat16

# gate permutation: keras [i f g o] -> ours [i f o g]
_PERM = np.concatenate([np.arange(0, 64), np.arange(96, 128), np.arange(64, 96)])

_CACHE = {}


def _build(Tn, abl=(), HAS_BIAS=True):
    nc = bacc.Bacc("TRN2", debug=False, num_devices=NCORES)
    x_d = nc.dram_tensor("x", [BS, Tn, F], BF, kind="ExternalInput")
    wenc2_d = nc.dram_tensor("wenc2", [128, 128], BF, kind="ExternalInput")
    benc2_d = nc.dram_tensor("benc2", [128, 1], DT, kind="ExternalInput")
    kern2_d = nc.dram_tensor("kern2", [128, 4 * H], BF, kind="ExternalInput")
    rec_d = nc.dram_tensor("rec", [H, 4 * H], BF, kind="ExternalInput")
    b4t_d = nc.dram_tensor("b4t", [4, H], BF, kind="ExternalInput")
    bind4_d = nc.dram_tensor("bind4", [4, 4 * BS], BF, kind="ExternalInput")
    sel4_d = nc.dram_tensor("sel4", [H, 4, 128], BF, kind="ExternalInput")
    attnw4_d = nc.dram_tensor("attnw4", [128, 128], BF, kind="ExternalInput")
    attnb4_d = nc.dram_tensor("attnb4", [128, 1], DT, kind="ExternalInput")
    attnu4_d = nc.dram_tensor("attnu4", [128, 4], DT, kind="ExternalInput")
    grp_d = nc.dram_tensor("grp", [4, 128], BF, kind="ExternalInput")
    ones4_d = nc.dram_tensor("ones4", [4, 1], BF, kind="ExternalInput")
    ones32_d = nc.dram_tensor("ones32", [1, H], DT, kind="ExternalInput")
    grpsel_d = nc.dram_tensor("grpsel", [128, H], DT, kind="ExternalInput")
    wdec0_d = nc.dram_tensor("wdec0", [H, 1], BF, kind="ExternalInput")
    wdec1_d = nc.dram_tensor("wdec1", [H, 1], DT, kind="ExternalInput")
    bdec_d = nc.dram_tensor("bdec", [BS, 1], DT, kind="ExternalInput")
    ident_d = nc.dram_tensor("ident", [128, 128], DT, kind="ExternalInput")
    out_d = nc.dram_tensor("out", [BS, 1], DT, kind="ExternalOutput")

    TC = 16  # timesteps per chunk
    nchunks = Tn // TC
    NQ = Tn // 4  # quads

    with tile.TileContext(nc) as tc:
        from contextlib import ExitStack

        ctx = ExitStack()
        with ctx:
            constp = ctx.enter_context(tc.tile_pool(name="const", bufs=1))
            xinp = ctx.enter_context(tc.tile_pool(name="xin", bufs=3))
            xtp = ctx.enter_context(tc.tile_pool(name="xt", bufs=3))
            encp = ctx.enter_context(tc.tile_pool(name="enc", bufs=3))
            sgp = ctx.enter_context(tc.tile_pool(name="sg", bufs=3))
            smp = ctx.enter_context(tc.tile_pool(name="sm", bufs=1))
            trps = ctx.enter_context(tc.tile_pool(name="trps", bufs=1, space="PSUM"))
            encps = ctx.enter_context(tc.tile_pool(name="encps", bufs=2, space="PSUM"))
            zps = ctx.enter_context(tc.tile_pool(name="zps", bufs=2, space="PSUM"))
            hqps = ctx.enter_context(tc.tile_pool(name="hqps", bufs=2, space="PSUM"))
            attps = ctx.enter_context(tc.tile_pool(name="attps", bufs=1, space="PSUM"))

            # ---- constants ----
            def cload(name, shape, dtype, src):
                t = constp.tile(shape, dtype, name=name)
                nc.sync.dma_start(t[:], src[tuple(slice(None) for _ in shape)])
                return t

            ident = cload("ident", [128, 128], DT, ident_d)
            wenc2 = cload("wenc2", [128, 128], BF, wenc2_d)
            benc2 = cload("benc2", [128, 1], DT, benc2_d)
            kern2 = cload("kern2", [128, 4 * H], BF, kern2_d)
            rec = cload("rec", [H, 4 * H], BF, rec_d)
            b4t = cload("b4t", [4, H], BF, b4t_d)
            bind4 = cload("bind4", [4, 4 * BS], BF, bind4_d)
            sel4 = cload("sel4", [H, 4, 128], BF, sel4_d)
            attnw4 = cload("attnw4", [128, 128], BF, attnw4_d)
            attnb4 = cload("attnb4", [128, 1], DT, attnb4_d)
            attnu4 = cload("attnu4", [128, 4], DT, attnu4_d)
            grp = cload("grp", [4, 128], BF, grp_d)
            ones4 = cload("ones4", [4, 1], BF, ones4_d)
            ones32 = cload("ones32", [1, H], DT, ones32_d)
            grpsel = cload("grpsel", [128, H], DT, grpsel_d)
            wdec0 = cload("wdec0", [H, 1], BF, wdec0_d)
            wdec1 = cload("wdec1", [H, 1], DT, wdec1_d)
            bdec = cload("bdec", [BS, 1], DT, bdec_d)

            # ---- persistent state ----
            h_cur = constp.tile([H, BS], BF)
            c_st = constp.tile([H, BS], BF)
            m_t = constp.tile([H, BS], BF)
            v_t = constp.tile([H, BS], BF)
            th_t = constp.tile([H, BS], BF)
            hstore = constp.tile([128, NQ * 128], BF)  # quad-packed h history
            lstore = constp.tile([4, NQ * 128], BF)  # attention logits
            acc = constp.tile([128, BS], DT)  # pooled accum (4-way split)
            nc.any.memset(h_cur[:], 0.0)
            nc.any.memset(c_st[:], 0.0)
            nc.any.memset(acc[:], 0.0)

            ztiles = {}
            hq_ps = {}
            encsb = {}

            def emit_dma(ck):
                xin = xinp.tile([128, TC, F], BF, tag="xin", name="xin")
                nc.sync.dma_start(xin[:], x_d[:, ck * TC : (ck + 1) * TC, :])
                return xin

            def emit_pre(xin, t):
                # pairs: DMA-transpose x[t,t+1] -> [128(2t,f), 128b]; blockdiag enc
                if t % 2 == 1:
                    return
                j = t % TC
                xts = xtp.tile([128, 128], BF, tag="xt", name="xts")
                nc.sync.dma_start_transpose(xts[:], xin[:, j : j + 2, :])
                eps = encps.tile([128, BS], DT, tag="encps", name="encps")
                nc.tensor.matmul(eps[:], wenc2[:], xts[:], start=True, stop=True,
                                 skip_group_check=True)
                eb = encp.tile([128, BS], BF, tag="enc", name="encsb")
                nc.scalar.activation(eb[:], eps[:], AF.Tanh, bias=benc2[:])
                encsb[t] = eb[0:64, :]
                encsb[t + 1] = eb[64:128, :]

            def emit_xg(t):
                # z psum tile [32, 4, 128]; bias + input-gate contributions
                zt = zps.tile([H, 4, BS], DT, tag="z", name="zt")
                ztiles[t] = zt
                if HAS_BIAS:
                    nc.tensor.matmul(zt[:], b4t[:], bind4[:], start=True,
                                     stop=False, skip_group_check=True)
                eb = encsb.pop(t)
                kb = 64 * (t % 2)
                for g in range(4):
                    nc.tensor.matmul(
                        zt[:, g, :],
                        kern2[kb : kb + 64, 32 * g : 32 * (g + 1)],
                        eb,
                        start=(not HAS_BIAS and g == 0),
                        stop=False,
                        skip_group_check=True,
                    )

            def step(t):
                q, j = divmod(t, 4)
                zt = ztiles.pop(t)
                # recurrence: g gate first so tanh can start early
                for g in (() if "norec" in abl else (0, 1, 2, 3)):
                    nc.tensor.matmul(
                        zt[:, g, :],
                        rec[:, 32 * g : 32 * (g + 1)],
                        h_cur[:],
                        start=False,
                        stop=(g == 3),
                        skip_group_check=True,
                    )
                s = sgp.tile([H, 4, BS], BF, tag="s", name="sgate")
                nc.scalar.activation(s[:], zt[:], AF.Sigmoid)
                # tanh(g) = 2*sigmoid(2g) - 1; g-gate weights pre-doubled.
                # m = i*tanh(g) = (2*s_g)*s_i - s_i
                tg = sgp.tile([H, BS], BF, tag="tg", name="tg")
                nc.vector.tensor_scalar(
                    tg[:], s[:, 3, :], 2.0, -1.0,
                    op0=mybir.AluOpType.mult, op1=mybir.AluOpType.add,
                )
                nc.vector.tensor_mul(m_t[:], tg[:], s[:, 0, :])
                nc.vector.tensor_mul(v_t[:], s[:, 1, :], c_st[:])
                nc.vector.tensor_add(c_st[:], m_t[:], v_t[:])
                if "noth" not in abl:
                    nc.scalar.activation(th_t[:], c_st[:], AF.Tanh)
                    nc.vector.tensor_mul(h_cur[:], s[:, 2, :], th_t[:])
                # pack h into quad psum via selector matmul
                if "nohq" in abl:
                    return
                if j == 0:
                    hq_ps[q] = hqps.tile([128, 128], DT, tag="hq", name="hq")
                nc.tensor.matmul(
                    hq_ps[q][:],
                    sel4[:, j, :],
                    h_cur[:],
                    start=(j == 0),
                    stop=(j == 3),
                    skip_group_check=True,
                )

            def pool_quad(q):
                hq = hq_ps.pop(q)
                hsl = hstore[:, 128 * q : 128 * (q + 1)]
                nc.vector.tensor_copy(hsl, hq[:])
                lps = attps.tile([128, 128], DT, tag="att", name="latps")
                nc.tensor.matmul(lps[:], attnw4[:], hsl, start=True, stop=True,
                                 skip_group_check=True)
                lat = sgp.tile([128, BS], DT, tag="lat", name="lat")
                nc.scalar.activation(lat[:], lps[:], AF.Tanh, bias=attnb4[:])
                gps = attps.tile([4, BS], DT, tag="att", name="gps")
                nc.tensor.matmul(gps[:], attnu4[:], lat[:], start=True, stop=True,
                                 skip_group_check=True)
                nc.vector.tensor_copy(lstore[:, 128 * q : 128 * (q + 1)], gps[:])

            # ---- main pipeline ----
            xin_cur = emit_dma(0)
            for j in range(TC):
                emit_pre(xin_cur, j)
            for t in range(4):
                emit_xg(t)
            xin_nxt = emit_dma(1) if nchunks > 1 else None

            for ck in range(nchunks):
                for j in range(TC):
                    t = ck * TC + j
                    step(t)
                    if t + 4 < Tn:
                        emit_xg(t + 4)
                    if t % 4 == 3 and "nopool" not in abl:
                        pool_quad(t // 4)
                    # phase A of chunk ck+1, spread across this chunk's steps
                    if ck + 1 < nchunks:
                        emit_pre(xin_nxt, (ck + 1) * TC + j)
                        if j == TC - 1:
                            xin_cur = xin_nxt
                            xin_nxt = emit_dma(ck + 2) if ck + 2 < nchunks else None

            # ---- post-pass: softmax pooling + decode ----
            estore = constp.tile([4, NQ * 128], BF)
            nc.scalar.activation(estore[:], lstore[:], AF.Exp)
            seps = encps.tile([1, BS], DT, tag="encps", name="seps")
            for q in range(NQ):
                esl = estore[:, 128 * q : 128 * (q + 1)]
                nc.tensor.matmul(
                    seps[:], ones4[:], esl, start=(q == 0), stop=(q == NQ - 1),
                    skip_group_check=True,
                )
                ebc = trps.tile([128, BS], DT, tag="tr", name="ebc")
                nc.tensor.matmul(ebc[:], grp[:], esl, start=True, stop=True,
                                 skip_group_check=True)
                wx = sgp.tile([128, BS], DT, tag="wx", name="wx")
                nc.vector.tensor_mul(wx[:], ebc[:], hstore[:, 128 * q : 128 * (q + 1)])
                nc.vector.tensor_add(acc[:], acc[:], wx[:])

            # pooled = (sum_g acc) / se ; out = sigmoid(h.Wd0 + pooled.Wd1 + b)
            pfold = attps.tile([H, BS], DT, tag="att", name="pfold")
            nc.tensor.matmul(pfold[:], grpsel[:], acc[:], start=True, stop=True,
                             skip_group_check=True)
            rse = smp.tile([1, BS], DT, name="rse")
            nc.vector.reciprocal(rse[:], seps[:])
            rbc = trps.tile([H, BS], DT, tag="tr", name="rbc")
            nc.tensor.matmul(rbc[:], ones32[:], rse[:], start=True, stop=True,
                             skip_group_check=True)
            rbcs = smp.tile([H, BS], DT, name="rbcs")
            nc.vector.tensor_copy(rbcs[:], rbc[:])
            pooled = smp.tile([H, BS], DT, name="pooled")
            nc.vector.tensor_mul(pooled[:], pfold[:], rbcs[:])
            po = encps.tile([BS, 1], DT, tag="encps", name="po")
            nc.tensor.matmul(po[:], h_cur[:], wdec0[:], start=True, stop=False,
                             skip_group_check=True)
            nc.tensor.matmul(po[:], pooled[:], wdec1[:], start=False,
                             stop=True, skip_group_check=True)
            osb = smp.tile([BS, 1], DT, name="osb")
            nc.scalar.activation(osb[:], po[:], AF.Sigmoid, bias=bdec[:])
            nc.sync.dma_start(out_d[:, :], osb[:])

    nc.compile()
    return nc


def _prep_shared(W_enc, b_enc, kernel, recurrent, bias, attn_W, attn_b, attn_u,
                 W_dec, b_dec):
    import ml_dtypes
    f32 = np.float32
    bf16 = ml_dtypes.bfloat16

    wenc2 = np.zeros((128, 128), np.float32)
    wenc2[0:64, 0:64] = W_enc
    wenc2[64:128, 64:128] = W_enc
    gscale = np.ones(128, np.float32)
    gscale[96:128] = 2.0  # tanh(g) = 2*sigmoid(2g)-1
    kern = np.ascontiguousarray(kernel[:, _PERM] * gscale).astype(f32)
    recp = np.ascontiguousarray(recurrent[:, _PERM] * gscale).astype(f32)
    biasp = (bias[_PERM] * gscale).astype(f32)  # [128] in [i f o g] order
    b4t = np.ascontiguousarray(biasp.reshape(4, H))  # [gate, h]
    bind4 = np.zeros((4, 4 * BS), f32)
    for g in range(4):
        bind4[g, g * BS : (g + 1) * BS] = 1.0
    sel4 = np.zeros((H, 4, 128), f32)
    for j in range(4):
        sel4[:, j, 32 * j : 32 * (j + 1)] = np.eye(H)
    attnw4 = np.zeros((128, 128), f32)
    attnb4 = np.zeros((128, 1), f32)
    attnu4 = np.zeros((128, 4), f32)
    for g in range(4):
        attnw4[32 * g : 32 * (g + 1), 32 * g : 32 * (g + 1)] = attn_W
        attnb4[32 * g : 32 * (g + 1), 0] = attn_b
        attnu4[32 * g : 32 * (g + 1), g] = attn_u
    grp = np.zeros((4, 128), f32)
    for g in range(4):
        grp[g, 32 * g : 32 * (g + 1)] = 1.0
    grpsel = np.zeros((128, H), f32)
    for g in range(4):
        grpsel[32 * g : 32 * (g + 1), :] = np.eye(H)
    return {
        "wenc2": wenc2.astype(bf16),
        "benc2": np.concatenate([b_enc, b_enc]).reshape(128, 1).astype(f32),
        "kern2": np.vstack([kern, kern]).astype(bf16),
        "rec": recp.astype(bf16),
        "b4t": b4t.astype(bf16),
        "bind4": bind4.astype(bf16),
        "sel4": sel4.astype(bf16),
        "attnw4": attnw4.astype(bf16),
        "attnb4": attnb4,
        "attnu4": attnu4,
        "grp": grp.astype(bf16),
        "ones4": np.ones((4, 1), f32).astype(bf16),
        "ones32": np.ones((1, H), f32),
        "grpsel": grpsel,
        "wdec0": np.ascontiguousarray(W_dec.astype(f32).reshape(2 * H, 1)[0:H]).astype(bf16),
        "wdec1": np.ascontiguousarray(W_dec.astype(f32).reshape(2 * H, 1)[H:]),
        "bdec": np.full((BS, 1), float(np.asarray(b_dec).reshape(-1)[0]), f32),
        "ident": np.eye(128, dtype=f32),
    }


def kernel(x, W_enc, b_enc, kernel, recurrent, bias, attn_W, attn_b, attn_u,
           W_dec, b_dec, _trace=False):
    import ml_dtypes
    x = np.asarray(x, np.float32).astype(ml_dtypes.bfloat16)
    Tn = x.shape[1]
    shared = _prep_shared(
        np.asarray(W_enc), np.asarray(b_enc), np.asarray(kernel),
        np.asarray(recurrent), np.asarray(bias), np.asarray(attn_W),
        np.asarray(attn_b), np.asarray(attn_u), np.asarray(W_dec),
        np.asarray(b_dec),
    )
    has_bias = bool(np.any(np.asarray(bias)))
    key = (Tn, has_bias)
    if key not in _CACHE:
        _CACHE[key] = _build(Tn, HAS_BIAS=has_bias)
    nc = _CACHE[key]
    in_maps = []
    for c in range(NCORES):
        m = dict(shared)
        m["x"] = np.ascontiguousarray(x[c * BS : (c + 1) * BS])
        in_maps.append(m)
    res = bass_utils.run_bass_kernel_spmd(
        nc, in_maps, core_ids=list(range(NCORES)), trace=_trace
    )
    out = np.concatenate([res.results[c]["out"] for c in range(NCORES)], axis=0)
    globals()["_LAST_EXEC_NS"] = getattr(res, "exec_time_ns", None)
    return out
